# revision 1
# baseline (speedup 1.0000x reference)
"""MemNet Trainium2 kernel (8 NeuronCores, SPMD, vocab-sharded logits).

Key structural insight: the reference initializes the memory bank M to zeros
and applies identical (slot-independent) updates whenever all slot scores tie.
At t=0 all scores are exactly 0 => the top-k softmax keeps ALL slots (scores >=
thr) => uniform 1/512 weights => every slot receives the identical update. By
induction all 512 slots remain bitwise identical forever, so the scan
degenerates exactly to a per-head rank-1 recurrence:

    rv_t = rv_{t-1} + m_{t-1}
    m_t  = m_{t-1} * (1 - sigmoid(er_t)/512) + sigmoid(a_t)/512 * wv_t

with (wv, er, a) = f([h_t, rv_{t-1}]) via W_iface.  (Verified numerically: a
full 512-slot float simulation keeps slot spread == 0.0 and matches this
recurrence to ~1e-7.)

The rv feedback is solved by fixed-point iteration (contraction ~0.1/iter);
each iteration is a batched matmul + sigmoid + a hardware linear scan
(tensor_tensor_scan) along the time axis.

Sharding: all 8 cores replicate the (small) transformer + recurrence; the
memory-bound logits projection [512,384]@[384,32000] is sharded 8-ways along
vocab; outputs are concatenated on the host. No collectives.
"""

import os
from contextlib import ExitStack

import numpy as np
import ml_dtypes

import concourse.bass as bass
import concourse.tile as tile
import concourse.mybir as mybir
from concourse.bass_utils import run_bass_kernel_spmd

F32 = mybir.dt.float32
F32R = mybir.dt.float32r
BF16 = mybir.dt.bfloat16

# model dims (hardcoded per problem spec)
VOCAB, D, FF, L, NH = 32000, 256, 1024, 2, 8
HD_ATT = D // NH
SLOTS, MD, MH, HD = 512, 128, 4, 32
B, T = 4, 128
DC = D + MD                      # 384 controller feature dim
NCORES = 8
VSH = VOCAB // NCORES            # 4000 vocab per core
VT = 500                         # vocab tile (<=512 psum bank)
NVT = VSH // VT                  # 8 tiles
K_ITERS = 4                      # fixed-point iterations for rv feedback

AF = mybir.ActivationFunctionType
ALU = mybir.AluOpType
AX = mybir.AxisListType


def build_nc():
    import os as _os
    PH = int(_os.environ.get("KERNEL_PHASE", "3"))  # 1=tf only, 2=+rec, 3=full
    nc = bass.Bass()

    # ---------------- DRAM parameters ----------------
    dp = nc.declare_dram_parameter
    x0_d = dp("x0", [T, B * D], F32, isOutput=False)             # embedded input, t-major
    wqk_d = dp("wqk", [L, 2, 128, 2 * D], BF16, isOutput=False)   # [l][kchunk][128, 512]
    wv_d = dp("wvw", [L, 2, 128, D], BF16, isOutput=False)
    wo_d = dp("wo", [L, 2, 128, D], BF16, isOutput=False)
    w1_d = dp("w1", [L, 2, 128, FF], BF16, isOutput=False)
    b1_d = dp("b1c", [L, 128, FF // 128], F32, isOutput=False)
    w2_d = dp("w2", [L, 8, 128, D], BF16, isOutput=False)
    b2_d = dp("b2r", [L, 1, D], BF16, isOutput=False)
    wifc_d = dp("wifc", [3, 128, 260], BF16, isOutput=False)      # [kchunk][128, wv|er|g]
    bwv_d = dp("bwv", [128, 1], F32, isOutput=False)             # b_wv / SLOTS
    ber_d = dp("ber", [128, 1], F32, isOutput=False)
    bg_d = dp("bg", [4, 1], F32, isOutput=False)
    wlg_d = dp("wlg", [3, 128, VSH], BF16, isOutput=False)        # per-core shard
    blg_d = dp("blg", [1, VSH], BF16, isOutput=False)
    mask4_d = dp("mask4", [128, 512], BF16, isOutput=False)      # causal 0/1 tril, x4
    idf_d = dp("idf", [128, 128], F32, isOutput=False)
    idb_d = dp("idb", [128, 128], BF16, isOutput=False)
    ones_d = dp("ones1", [1, 128], BF16, isOutput=False)
    blk_d = dp("blkmask", [4, 128], BF16, isOutput=False)
    czb_d = dp("czb", [128, 2], F32, isOutput=False)  # col0=0.0, col1=1e-5
    out_d = dp("out", [B, T, VSH], F32, isOutput=True)

    with tile.TileContext(nc) as tc, ExitStack() as ctx:
        pers = ctx.enter_context(tc.tile_pool(name="pers", bufs=1))
        work = ctx.enter_context(tc.tile_pool(name="work", bufs=2))

        def P(shape, dt, tag):
            return pers.tile(shape, dt, tag=tag, name=tag)

        # ------------- load weights / constants to SBUF -------------
        def load(dram_ap, shape, dt, tag):
            t_ = P(shape, dt, tag)
            nc.sync.dma_start(t_[:], dram_ap)
            return t_

        wqk_sb = [[load(wqk_d[l, c], [128, 512], BF16, f"wqk{l}{c}") for c in range(2)] for l in range(L)]
        wv_sb = [[load(wv_d[l, c], [128, D], BF16, f"wv{l}{c}") for c in range(2)] for l in range(L)]
        wo_sb = [[load(wo_d[l, c], [128, D], BF16, f"wo{l}{c}") for c in range(2)] for l in range(L)]
        w1_sb = [[load(w1_d[l, c], [128, FF], BF16, f"w1{l}{c}") for c in range(2)] for l in range(L)]
        b1_sb = [load(b1_d[l], [128, FF // 128], F32, f"b1{l}") for l in range(L)]
        w2_sb = [[load(w2_d[l, c], [128, D], BF16, f"w2{l}{c}") for c in range(8)] for l in range(L)]
        b2_sb = [load(b2_d[l], [1, D], BF16, f"b2{l}") for l in range(L)]
        wifc_sb = [load(wifc_d[c], [128, 260], BF16, f"wifc{c}") for c in range(3)]
        bwv_sb = load(bwv_d[:, :], [128, 1], F32, "bwv")
        ber_sb = load(ber_d[:, :], [128, 1], F32, "ber")
        bg_sb = load(bg_d[:, :], [4, 1], F32, "bg")
        blg_sb = load(blg_d[:, :], [1, VSH], BF16, "blg")
        mask4_sb = load(mask4_d[:, :], [128, 512], BF16, "mask4")
        idf_sb = load(idf_d[:, :], [128, 128], F32, "idf")
        idb_sb = load(idb_d[:, :], [128, 128], BF16, "idb")
        ones_sb = load(ones_d[:, :], [1, 128], BF16, "ones1")
        blk_sb = load(blk_d[:, :], [4, 128], BF16, "blk")
        czb_sb = load(czb_d[:, :], [128, 2], F32, "czb")

        # residual stream x: [128 t, 4*256] f32
        x_sb = P([128, B * D], F32, "x")
        nc.sync.dma_start(x_sb[:], x0_d[:, :])

        # controller-state transposed C^T chunks [128, (b,t)=512]
        ct = [P([128, B * T], BF16, f"ct{c}") for c in range(3)]

        # ------------- helpers -------------
        def layernorm_std(x_in):
            """standardize (x-mu)/sqrt(var+eps) over d per sample; [128, B*D]"""
            ssum = work.tile([128, B], F32, tag="ln_ss", name="ln_ss")
            nc.vector.tensor_reduce(ssum[:], x_in[:].rearrange("p (b d) -> p b d", d=D),
                                    axis=AX.X, op=ALU.add, negate=True)
            mun = work.tile([128, B], F32, tag="ln_mu", name="ln_mu")
            nc.vector.tensor_scalar_mul(mun[:], ssum[:], 1.0 / D)
            xc = work.tile([128, B * D], F32, tag="ln_xc", name="ln_xc", bufs=1)
            for b in range(B):
                nc.vector.tensor_scalar_add(xc[:, b * D:(b + 1) * D],
                                            x_in[:, b * D:(b + 1) * D], mun[:, b:b + 1])
            sq = work.tile([128, B * D], F32, tag="ln_sq", name="ln_sq", bufs=1)
            nc.vector.tensor_mul(sq[:], xc[:], xc[:])
            vs = work.tile([128, B], F32, tag="ln_vs", name="ln_vs")
            nc.vector.tensor_reduce(vs[:], sq[:].rearrange("p (b d) -> p b d", d=D),
                                    axis=AX.X, op=ALU.add)
            sd = work.tile([128, B], F32, tag="ln_sd", name="ln_sd")
            nc.scalar.activation(sd[:], vs[:], AF.Sqrt, bias=czb_sb[:, 1:2], scale=1.0 / D)
            rstd = work.tile([128, B], F32, tag="ln_rs", name="ln_rs")
            nc.vector.reciprocal(rstd[:], sd[:])
            xn = work.tile([128, B * D], F32, tag="ln_xn", name="ln_xn", bufs=1)
            for b in range(B):
                nc.vector.tensor_scalar_mul(xn[:, b * D:(b + 1) * D],
                                            xc[:, b * D:(b + 1) * D], rstd[:, b:b + 1])
            return xn

        def transpose_to(pool, xn, dest_tiles):
            """xn [128 t, B*256 d] -> dest_tiles[c] [128 d, B*128 t] f32 (c=0,1)"""
            for b in range(B):
                for c in range(2):
                    pt = pool.tile([128, 128], F32, tag="tp", name="tp", bufs=2)
                    nc.tensor.transpose(pt[:], xn[:, b * D + c * 128: b * D + (c + 1) * 128], idf_sb[:])
                    nc.vector.tensor_copy(dest_tiles[c][:, b * 128:(b + 1) * 128], pt[:])

        # ------------- transformer layers -------------
        for l in range(L):
            h1 = layernorm_std(x_sb)
            h1t = [work.tile([128, B * T], BF16, tag=f"ht{c}", name=f"ht{c}", bufs=1) for c in range(2)]
            q_bf = [work.tile([32, B * T], BF16, tag=f"qh{h}", name=f"qh{h}", bufs=1) for h in range(NH)]
            k_bf = [work.tile([32, B * T], BF16, tag=f"kh{h}", name=f"kh{h}", bufs=1) for h in range(NH)]
            v_bf = [work.tile([128, D], BF16, tag=f"vb{b}", name=f"vb{b}") for b in range(B)]
            with tc.tile_pool(name=f"psA{l}", bufs=1, space="PSUM") as psA:
                transpose_to(psA, h1, h1t)
                # qk^T feature-major bf16, split into per-head [32, 512] tiles
                for m in range(4):
                    qkp = psA.tile([128, B * T], F32, tag="qkp", name="qkp", bufs=2)
                    for c in range(2):
                        nc.tensor.matmul(qkp[:], wqk_sb[l][c][:, m * 128:(m + 1) * 128],
                                         h1t[c][:], start=(c == 0), stop=(c == 1))
                    for q4 in range(4):
                        h = (m % 2) * 4 + q4
                        dst = (q_bf if m < 2 else k_bf)[h]
                        if q4 % 2 == 0:
                            nc.scalar.activation(dst[:], qkp[q4 * 32:(q4 + 1) * 32, :], AF.Copy)
                        else:
                            nc.vector.tensor_copy(dst[:], qkp[q4 * 32:(q4 + 1) * 32, :])
                # v time-major bf16 per sample [128 t, 256 d]
                for b in range(B):
                    vp = psA.tile([128, D], F32, tag="vp", name="vp", bufs=2)
                    for c in range(2):
                        nc.tensor.matmul(vp[:], h1t[c][:, b * 128:(b + 1) * 128],
                                         wv_sb[l][c][:], start=(c == 0), stop=(c == 1))
                    nc.vector.tensor_copy(v_bf[b][:], vp[:])

            # attention; oT accumulated per head in [32, (b,t)] psum tiles
            ot_sb = [work.tile([128, B * T], BF16, tag=f"ot{mt}", name=f"ot{mt}", bufs=1) for mt in range(2)]
            with tc.tile_pool(name=f"psB{l}", bufs=1, space="PSUM") as psB:
                for mt in range(2):
                    otp = [psB.tile([32, B * T], F32, tag=f"otp{j}", name=f"otp{j}", bufs=1)
                           for j in range(4)]
                    for b in range(B):
                        scp = psB.tile([128, 512], F32, tag="scp", name="scp", bufs=2)
                        for j in range(4):
                            h = mt * 4 + j
                            cols = slice(b * 128, (b + 1) * 128)
                            nc.tensor.matmul(scp[:, j * 128:(j + 1) * 128],
                                             q_bf[h][:, cols], k_bf[h][:, cols],
                                             start=True, stop=True)
                        att = work.tile([128, 512], BF16, tag="att", name="att")
                        nc.scalar.activation(att[:], scp[:], AF.Exp, bias=czb_sb[:, 0:1], scale=1.0)
                        amask = work.tile([128, 512], BF16, tag="amask", name="amask")
                        nc.vector.tensor_mul(amask[:], att[:], mask4_sb[:])
                        rs = work.tile([128, 4], F32, tag="rsum", name="rsum")
                        nc.vector.tensor_reduce(rs[:], amask[:].rearrange("p (j t) -> p j t", t=128),
                                                axis=AX.X, op=ALU.add)
                        rr = work.tile([128, 4], F32, tag="rrec", name="rrec")
                        nc.vector.reciprocal(rr[:], rs[:])
                        attn = work.tile([128, 512], BF16, tag="attn", name="attn")
                        for j in range(4):
                            nc.vector.tensor_scalar_mul(attn[:, j * 128:(j + 1) * 128],
                                                        amask[:, j * 128:(j + 1) * 128], rr[:, j:j + 1])
                        atp = psB.tile([128, 512], BF16, tag="atp", name="atp", bufs=2)
                        for j in range(4):
                            nc.tensor.transpose(atp[:, j * 128:(j + 1) * 128],
                                                attn[:, j * 128:(j + 1) * 128], idb_sb[:])
                        attb = work.tile([128, 512], BF16, tag="attb", name="attb")
                        if (b + mt) % 2 == 0:
                            nc.scalar.activation(attb[:], atp[:], AF.Copy)
                        else:
                            nc.vector.tensor_copy(attb[:], atp[:])
                        for j in range(4):
                            h = mt * 4 + j
                            nc.tensor.matmul(otp[j][:, b * 128:(b + 1) * 128],
                                             v_bf[b][:, h * 32:(h + 1) * 32],
                                             attb[:, j * 128:(j + 1) * 128],
                                             start=True, stop=True)
                    for j in range(4):
                        dst = ot_sb[mt][j * 32:(j + 1) * 32, :]
                        if j % 2 == 0:
                            nc.scalar.activation(dst, otp[j][:], AF.Copy)
                        else:
                            nc.vector.tensor_copy(dst, otp[j][:])
            with tc.tile_pool(name=f"psW{l}", bufs=1, space="PSUM") as psW:
                for b in range(B):
                    yp = psW.tile([128, D], F32, tag="yp", name="yp", bufs=2)
                    for c in range(2):
                        nc.tensor.matmul(yp[:], ot_sb[c][:, b * 128:(b + 1) * 128],
                                         wo_sb[l][c][:], start=(c == 0), stop=(c == 1))
                    nc.vector.tensor_add(x_sb[:, b * D:(b + 1) * D], x_sb[:, b * D:(b + 1) * D], yp[:])

            # FF
            h2 = layernorm_std(x_sb)
            h2t = [work.tile([128, B * T], BF16, tag=f"ht{c}", name=f"ht{c}", bufs=1) for c in range(2)]
            ut = [work.tile([128, B * T], BF16, tag=f"ut{ft}", name=f"ut{ft}", bufs=1) for ft in range(8)]
            with tc.tile_pool(name=f"psC{l}", bufs=1, space="PSUM") as psC:
                transpose_to(psC, h2, h2t)
                for ft in range(8):
                    up = psC.tile([128, B * T], F32, tag="up", name="up", bufs=3)
                    for c in range(2):
                        nc.tensor.matmul(up[:], w1_sb[l][c][:, ft * 128:(ft + 1) * 128],
                                         h2t[c][:], start=(c == 0), stop=(c == 1))
                    # tanh-approx gelu (0.5 factor folded into W2):
                    # ut = u * (1 + tanh(0.79788456*(u + 0.044715 u^3)))
                    uu = work.tile([128, B * T], F32, tag="gl_u", name="gl_u", bufs=2)
                    nc.vector.tensor_scalar_add(uu[:], up[:], b1_sb[l][:, ft:ft + 1])
                    u2 = work.tile([128, B * T], F32, tag="gl_u2", name="gl_u2", bufs=2)
                    nc.vector.tensor_mul(u2[:], uu[:], uu[:])
                    ga = work.tile([128, B * T], F32, tag="gl_a", name="gl_a", bufs=2)
                    nc.vector.tensor_scalar(ga[:], u2[:], 0.044715, 1.0, op0=ALU.mult, op1=ALU.add)
                    gb = work.tile([128, B * T], F32, tag="gl_b", name="gl_b", bufs=2)
                    nc.vector.scalar_tensor_tensor(gb[:], ga[:], 0.7978845608028654, uu[:],
                                                   op0=ALU.mult, op1=ALU.mult)
                    gt = work.tile([128, B * T], F32, tag="gl_t", name="gl_t", bufs=2)
                    nc.scalar.activation(gt[:], gb[:], AF.Tanh, bias=czb_sb[:, 0:1], scale=1.0)
                    nc.vector.scalar_tensor_tensor(ut[ft][:], gt[:], 1.0, uu[:],
                                                   op0=ALU.add, op1=ALU.mult)
                for b in range(B):
                    y2 = psC.tile([128, D], F32, tag="y2p", name="y2p", bufs=2)
                    for c in range(8):
                        nc.tensor.matmul(y2[:], ut[c][:, b * 128:(b + 1) * 128],
                                         w2_sb[l][c][:], start=(c == 0), stop=False)
                    nc.tensor.matmul(y2[:], ones_sb[:], b2_sb[l][:], start=False, stop=True)
                    nc.vector.tensor_add(x_sb[:, b * D:(b + 1) * D], x_sb[:, b * D:(b + 1) * D], y2[:])

        # final LN -> C^T chunks 0,1
        hn = layernorm_std(x_sb)
        with tc.tile_pool(name="psF", bufs=1, space="PSUM") as psF:
            transpose_to(psF, hn, ct)

        # ------------- degenerate memory recurrence (fixed point) -------------
        nc.vector.memset(ct[2][:], 0.0)
        K_IT = K_ITERS if PH >= 2 else 0
        with tc.tile_pool(name="psR", bufs=1, space="PSUM") as psR:
            for it in range(K_IT):
                nchunk = 2 if it == 0 else 3
                wvp = psR.tile([128, B * T], F32, tag="wvp", name="wvp", bufs=1)
                erp = psR.tile([128, B * T], F32, tag="erp", name="erp", bufs=1)
                gp = psR.tile([4, B * T], F32, tag="gp", name="gp", bufs=1)
                for c in range(nchunk):
                    st, sp = (c == 0), (c == nchunk - 1)
                    nc.tensor.matmul(wvp[:], wifc_sb[c][:, 0:128], ct[c][:], start=st, stop=sp)
                    nc.tensor.matmul(erp[:], wifc_sb[c][:, 128:256], ct[c][:], start=st, stop=sp)
                    nc.tensor.matmul(gp[:], wifc_sb[c][:, 256:260], ct[c][:], start=st, stop=sp)
                se = work.tile([128, B * T], F32, tag="se", name="se", bufs=1)
                nc.scalar.activation(se[:], erp[:], AF.Sigmoid, bias=ber_sb[:, 0:1], scale=1.0)
                sa = work.tile([4, B * T], BF16, tag="sa", name="sa")
                nc.scalar.activation(sa[:], gp[:], AF.Sigmoid, bias=bg_sb[:, 0:1], scale=1.0)
                s_sb = work.tile([128, B * T], F32, tag="ssb", name="ssb", bufs=1)
                nc.vector.tensor_scalar(s_sb[:], se[:], -1.0 / SLOTS, 1.0, op0=ALU.mult, op1=ALU.add)
                wvb = work.tile([128, B * T], F32, tag="wvb", name="wvb", bufs=1)
                nc.vector.tensor_scalar(wvb[:], wvp[:], 1.0 / SLOTS, bwv_sb[:, 0:1],
                                        op0=ALU.mult, op1=ALU.add)
                sabc = psR.tile([128, B * T], F32, tag="sabc", name="sabc", bufs=1)
                nc.tensor.matmul(sabc[:], blk_sb[:], sa[:], start=True, stop=True)
                u_sb = work.tile([128, B * T], F32, tag="usb", name="usb", bufs=1)
                nc.vector.tensor_mul(u_sb[:], wvb[:], sabc[:])
                msc = work.tile([128, B * T], F32, tag="msc", name="msc", bufs=1)
                pcs = work.tile([128, B * T], F32, tag="pcs", name="pcs", bufs=1)
                for b in range(B):
                    cs = slice(b * 128, (b + 1) * 128)
                    nc.vector.tensor_tensor_scan(msc[:, cs], s_sb[:, cs], u_sb[:, cs],
                                                 0.0, op0=ALU.mult, op1=ALU.add)
                    nc.vector.tensor_tensor_scan(pcs[:, cs], msc[:, cs], msc[:, cs],
                                                 0.0, op0=ALU.add, op1=ALU.bypass)
                    nc.vector.memset(ct[2][:, b * 128: b * 128 + 2], 0.0)
                    nc.vector.tensor_copy(ct[2][:, b * 128 + 2:(b + 1) * 128], pcs[:, b * 128: b * 128 + 126])

        # ------------- logits = C @ W_lg + b_lg (vocab shard) -------------
        if PH < 3:
            nc.sync.dma_start(out_d[0, :, 0:B * D], x_sb[:])
            nc.sync.dma_start(out_d[1, :, 0:B * T // 2], ct[2][:].bitcast(F32)[:, :B * T // 2])
        with tc.tile_pool(name="psL", bufs=1, space="PSUM") as psL:
            for vt in range(NVT if PH >= 3 else 0):
                vcols = slice(vt * VT, (vt + 1) * VT)
                wlgt = [work.tile([128, VT], BF16, tag=f"wlgt{c}", name=f"wlgt{c}", bufs=2)
                        for c in range(3)]
                for c in range(3):
                    nc.sync.dma_start(wlgt[c][:], wlg_d[c][:, vcols])
                for b in range(B):
                    lg = psL.tile([128, VT], F32, tag="lg", name="lg", bufs=4)
                    for c in range(3):
                        nc.tensor.matmul(lg[:], ct[c][:, b * 128:(b + 1) * 128],
                                         wlgt[c][:], start=(c == 0), stop=False)
                    nc.tensor.matmul(lg[:], ones_sb[:], blg_sb[:, vcols], start=False, stop=True)
                    lgs = work.tile([128, VT], F32, tag="lgs", name="lgs")
                    if (b + vt) % 2 == 0:
                        nc.scalar.activation(lgs[:], lg[:], AF.Copy)
                    else:
                        nc.vector.tensor_copy(lgs[:], lg[:])
                    nc.sync.dma_start(out_d[b, :, vcols], lgs[:])

    _split_excess_waits(nc, maxw=1)
    return nc


def _split_excess_waits(nc, maxw=1):
    """The TPB instruction encodings carry a limited number of sync-wait
    slots; walrus rejects instructions with more ("Too many sync wait
    commands").  Move overflow waits onto same-engine NoOps inserted just
    before the instruction (program order on the sequencer preserves the
    blocking semantics)."""
    n_id = [0]
    for fn in nc.m.functions:
        for blk in fn.blocks:
            insts = blk.instructions
            idx = 0
            while idx < len(insts):
                ins = insts[idx]
                si = ins.sync_info
                if si is not None and len(si.on_wait) > maxw:
                    waits = list(si.on_wait)
                    keep = waits[-maxw:]
                    overflow = waits[:-maxw]
                    for j in range(0, len(overflow), max(maxw, 1)):
                        chunk = overflow[j:j + max(maxw, 1)]
                        nop = mybir.InstNoOp(name=nc.get_next_instruction_name(), ins=[], outs=[])
                        n_id[0] += 1
                        nop.engine = ins.engine
                        nop.sync_info = mybir.SyncInfo(on_wait=chunk, on_update=[])
                        nc.register_instruction(nop)
                        insts.insert(idx, nop)
                        idx += 1
                    si.on_wait = keep
                idx += 1


# ---------------- host side ----------------
_NC_CACHE = {}


def _get_nc():
    if "nc" not in _NC_CACHE:
        _NC_CACHE["nc"] = build_nc()
    return _NC_CACHE["nc"]


def prepare_in_maps(input_seq, tok_emb, pos_emb, Wqkv, Wo, ln1_g, ln1_b, ln2_g, ln2_b,
                    W1, b1, W2, b2, lnf_g, lnf_b, W_logits, b_logits, W_iface, b_iface,
                    beta_read, beta_write):
    f = np.float32
    input_seq = np.asarray(input_seq)
    tok_emb = np.asarray(tok_emb, f)
    pos_emb = np.asarray(pos_emb, f)
    Wqkv = np.asarray(Wqkv, f); Wo = np.asarray(Wo, f)
    ln1_g = np.asarray(ln1_g, f); ln1_b = np.asarray(ln1_b, f)
    ln2_g = np.asarray(ln2_g, f); ln2_b = np.asarray(ln2_b, f)
    W1 = np.asarray(W1, f); b1 = np.asarray(b1, f)
    W2 = np.asarray(W2, f); b2 = np.asarray(b2, f)
    lnf_g = np.asarray(lnf_g, f); lnf_b = np.asarray(lnf_b, f)
    W_logits = np.asarray(W_logits, f); b_logits = np.asarray(b_logits, f)
    W_iface = np.asarray(W_iface, f); b_iface = np.asarray(b_iface, f)

    # embedding (input prep)
    x0 = (tok_emb[input_seq] + pos_emb[:T]).astype(f)            # [B, T, D]
    x0 = np.ascontiguousarray(x0.transpose(1, 0, 2).reshape(T, B * D))

    # LN gamma folds (beta folds where a bias path exists)
    wqk = np.ascontiguousarray((ln1_g[:, :, None] * Wqkv[:, :, :2 * D]))
    wqk[:, :, :D] *= f(1.0 / np.sqrt(HD_ATT))
    wqk = wqk.reshape(L, 2, 128, 2 * D)
    wvw = (ln1_g[:, :, None] * Wqkv[:, :, 2 * D:]).reshape(L, 2, 128, D)
    wo_r = Wo.reshape(L, 2, 128, D)
    w1 = (ln2_g[:, :, None] * W1).reshape(L, 2, 128, FF)
    b1c = np.ascontiguousarray(
        (b1 + np.einsum("ld,ldf->lf", ln2_b, W1)).reshape(L, 8, 128).transpose(0, 2, 1))
    w2 = (0.5 * W2).reshape(L, 8, 128, D)
    b2r = b2.reshape(L, 1, D)

    # W_iface columns: per head block h*(4*HD+1): [rk, wk, wv, er, gate]
    Wif = W_iface.copy()
    Wif[:D] *= lnf_g[:, None]
    bif_full = b_iface + lnf_b @ W_iface[:D]
    cols_wv, cols_er, cols_g = [], [], []
    for h in range(MH):
        base = h * (4 * HD + 1)
        cols_wv += list(range(base + 2 * HD, base + 3 * HD))
        cols_er += list(range(base + 3 * HD, base + 4 * HD))
        cols_g.append(base + 4 * HD)
    wifc = np.ascontiguousarray(
        np.concatenate([Wif[:, cols_wv], Wif[:, cols_er], Wif[:, cols_g]], axis=1)
    ).reshape(3, 128, 260)
    bwv = (bif_full[cols_wv] / SLOTS).reshape(128, 1).astype(f)
    ber = bif_full[cols_er].reshape(128, 1).astype(f)
    bg = bif_full[cols_g].reshape(4, 1).astype(f)

    Wlg = W_logits.copy()
    Wlg[:D] *= lnf_g[:, None]
    blg_full = b_logits + lnf_b @ W_logits[:D]

    mask1 = np.tril(np.ones((T, T), f))
    mask4 = np.tile(mask1, (1, 4)).astype(ml_dtypes.bfloat16)
    idf = np.eye(128, dtype=f)
    idb = np.eye(128, dtype=ml_dtypes.bfloat16)
    bf = ml_dtypes.bfloat16
    ones1 = np.ones((1, 128), bf)
    blk = np.zeros((4, 128), bf)
    for h in range(MH):
        blk[h, h * HD:(h + 1) * HD] = 1.0

    shared = {
        "x0": x0, "wqk": wqk.astype(bf), "wvw": wvw.astype(bf), "wo": wo_r.astype(bf),
        "w1": w1.astype(bf), "b1c": b1c.astype(f), "w2": w2.astype(bf), "b2r": b2r.astype(bf),
        "wifc": wifc.astype(bf), "bwv": bwv, "ber": ber, "bg": bg,
        "mask4": mask4, "idf": idf, "idb": idb, "ones1": ones1, "blkmask": blk,
        "czb": np.concatenate([np.zeros((128, 1), f), np.full((128, 1), 1e-5, f)], axis=1),
    }
    in_maps = []
    for c in range(NCORES):
        m = dict(shared)
        sl = slice(c * VSH, (c + 1) * VSH)
        m["wlg"] = np.ascontiguousarray(Wlg[:, sl]).reshape(3, 128, VSH).astype(bf)
        m["blg"] = blg_full[sl].reshape(1, VSH).astype(bf)
        in_maps.append(m)
    return in_maps


def kernel(**inputs):
    in_maps = prepare_in_maps(**inputs)
    nc = _get_nc()
    res = run_bass_kernel_spmd(nc, in_maps, list(range(NCORES))).results
    return np.concatenate([res[c]["out"] for c in range(NCORES)], axis=-1)



# revision 15
# speedup vs baseline: 1.5917x; 1.5917x over previous
"""MemNet Trainium2 kernel (8 NeuronCores, SPMD, vocab-sharded logits).

Key structural insight: the reference initializes the memory bank M to zeros
and applies identical (slot-independent) updates whenever all slot scores tie.
At t=0 all scores are exactly 0 => the top-k softmax keeps ALL slots (scores >=
thr) => uniform 1/512 weights => every slot receives the identical update. By
induction all 512 slots remain bitwise identical forever, so the scan
degenerates exactly to a per-head rank-1 recurrence:

    rv_t = rv_{t-1} + m_{t-1}
    m_t  = m_{t-1} * (1 - sigmoid(er_t)/512) + sigmoid(a_t)/512 * wv_t

with (wv, er, a) = f([h_t, rv_{t-1}]) via W_iface.  The rv feedback is solved
by fixed-point iteration (contraction ~0.1/iter).

Performance structure (cost-model driven):
  - all weights packed into 3 DRAM tensors, loaded with a handful of large
    DMAs spread across engines (SP carries the non-critical ones)
  - LN: sum/sumsq reduces + fused (x-mu)*rstd dual-scalar ops, rstd via
    ALU pow(-0.5) (no Act table for Sqrt); LN output bf16
  - attention: exp on Act (exp_and_others table also hosts tanh for gelu),
    mask-mult + rowsum on DVE, per-head normalize spread across engines
  - gelu composite: psum+bias copy on Pool, polynomial on DVE, tanh on Act
  - elementwise work spread across DVE / Act(Copy) / Pool round-robin
  - recurrence: one segmented scan per scan-kind (boundary-mask trick)
  - logits: vocab-sharded; h-chunk matmuls + bias (host-broadcast, added
    during psum drain) run overlapped with the recurrence; only the small
    rv-chunk matmul + drain + bf16 output DMA trail the fixed point
"""

import os
from contextlib import ExitStack

import numpy as np
import ml_dtypes

import concourse.bass as bass
import concourse.tile as tile
import concourse.mybir as mybir
from concourse.bass_utils import run_bass_kernel_spmd

F32 = mybir.dt.float32
BF16 = mybir.dt.bfloat16

# model dims (hardcoded per problem spec)
VOCAB, D, FF, L, NH = 32000, 256, 1024, 2, 8
HD_ATT = D // NH
SLOTS, MD, MH, HD = 512, 128, 4, 32
B, T = 4, 128
DC = D + MD                      # 384 controller feature dim
NCORES = 8
VSH = VOCAB // NCORES            # 4000 vocab per core
VT = 500                         # vocab tile (<=512 psum bank)
NVT = VSH // VT                  # 8 tiles
K_ITERS = 4                      # fixed-point iterations for rv feedback
BT = B * T

AF = mybir.ActivationFunctionType
ALU = mybir.AluOpType
AX = mybir.AxisListType

# ---- packed weight layouts (shared between host prep and device code) ----
# bf16 pack [128, WCOLS]
_woff = {}
_c = 0
def _walloc(name, cols):
    global _c
    _woff[name] = _c
    _c += cols
    return _woff[name]

for l in range(L):
    for c in range(2):
        _walloc(f"wqk{l}{c}", 2 * D)       # [128, 512]
for l in range(L):
    for c in range(2):
        _walloc(f"wv{l}{c}", D)
for l in range(L):
    for c in range(2):
        _walloc(f"wo{l}{c}", D)
for l in range(L):
    for c in range(2):
        _walloc(f"w1{l}{c}", FF)
for l in range(L):
    for c in range(8):
        _walloc(f"w2{l}{c}", D)
for c in range(3):
    _walloc(f"wifc{c}", 260)
_walloc("mask4", 512)                       # causal 0/1 tril x4 (bf16)
_walloc("bmask", 512)                       # 1 except 0 at b starts
_walloc("idb", 128)                         # identity bf16
_walloc("ones1", 128)                       # row 0: ones [1,128]
_walloc("blk", 128)                         # rows 0-3: head blocks [4,128]
for l in range(L):
    _walloc(f"b2{l}", D)                    # row 0: [1, 256]
_walloc("biasbc", VSH)                      # logits bias broadcast [128, VSH]
WCOLS = _c

# f32 pack [128, FCOLS]
_foff = {}
_f = 0
def _falloc(name, cols):
    global _f
    _foff[name] = _f
    _f += cols
    return _foff[name]

_falloc("x0", B * D)                        # [128 t, B*D]
for l in range(L):
    _falloc(f"b1c{l}", 8)                   # [128, 8] per-ft bias
_falloc("bmaskf", 512)
_falloc("bwv", 1)
_falloc("ber", 1)
_falloc("bg", 1)                            # rows 0-3
FCOLS = _f

# wpk DMA split points (col ranges -> engine)
_WSPLIT0 = _woff["w10" + "0"] if False else _woff["w100"]   # start of w1
_WSPLIT1 = _woff["wifc0"]                                   # start of misc


def build_nc():
    PH = int(os.environ.get("KERNEL_PHASE", "3"))  # 1=tf only, 2=+rec, 3=full
    nc = bass.Bass()

    dp = nc.declare_dram_parameter
    fpk_d = dp("fpk", [128, FCOLS], F32, isOutput=False)
    wpk_d = dp("wpk", [128, WCOLS], BF16, isOutput=False)
    wlg_d = dp("wlg", [3, 128, VSH], BF16, isOutput=False)   # per-core shard
    out_d = dp("out", [B, T, VSH], BF16, isOutput=True)

    with tile.TileContext(nc) as tc, ExitStack() as ctx:
        pers = ctx.enter_context(tc.tile_pool(name="pers", bufs=1))
        work = ctx.enter_context(tc.tile_pool(name="work", bufs=2))

        fpk = pers.tile([128, FCOLS], F32, tag="fpk", name="fpk")
        wpk = pers.tile([128, WCOLS], BF16, tag="wpk", name="wpk")
        wlg_sb = [pers.tile([128, VSH], BF16, tag=f"wlg{c}", name=f"wlg{c}")
                  for c in range(3)]

        def wv_(name, cols, rows=None):
            off = _woff[name]
            if rows is None:
                return wpk[:, off:off + cols]
            return wpk[rows[0]:rows[1], off:off + cols]

        wqk_sb = [[wv_(f"wqk{l}{c}", 2 * D) for c in range(2)] for l in range(L)]
        wvw_sb = [[wv_(f"wv{l}{c}", D) for c in range(2)] for l in range(L)]
        wo_sb = [[wv_(f"wo{l}{c}", D) for c in range(2)] for l in range(L)]
        w1_sb = [[wv_(f"w1{l}{c}", FF) for c in range(2)] for l in range(L)]
        w2_sb = [[wv_(f"w2{l}{c}", D) for c in range(8)] for l in range(L)]
        wifc_sb = [wv_(f"wifc{c}", 260) for c in range(3)]
        mask4_sb = wv_("mask4", 512)
        bmask_sb = wv_("bmask", 512)
        idb_sb = wv_("idb", 128)
        ones_sb = wv_("ones1", 128, rows=(0, 1))
        blk_sb = wv_("blk", 128, rows=(0, 4))
        b2_sb = [wv_(f"b2{l}", D, rows=(0, 1)) for l in range(L)]
        biasbc_sb = wv_("biasbc", VSH)

        x_sb = fpk[:, _foff["x0"]:_foff["x0"] + B * D]      # residual stream f32
        b1c_sb = [fpk[:, _foff[f"b1c{l}"]:_foff[f"b1c{l}"] + 8] for l in range(L)]
        bwv_sb = fpk[:, _foff["bwv"]:_foff["bwv"] + 1]
        ber_sb = fpk[:, _foff["ber"]:_foff["ber"] + 1]
        bg_sb = fpk[0:4, _foff["bg"]:_foff["bg"] + 1]
        bmaskf_sb = fpk[:, _foff["bmaskf"]:_foff["bmaskf"] + 512]

        # ---------------- startup DMAs ----------------
        # SP: f32 pack (critical: x0), then qkv/wo weights, then the
        # late-needed logits tensors. Act: w1. Pool: w2 + misc.
        nc.sync.dma_start(fpk[:], fpk_d[:, :])
        nc.sync.dma_start(wpk[:, 0:_WSPLIT0], wpk_d[:, 0:_WSPLIT0])
        nc.scalar.dma_start(wpk[:, _WSPLIT0:_woff["w200"]],
                            wpk_d[:, _WSPLIT0:_woff["w200"]])
        nc.gpsimd.dma_start(wpk[:, _woff["w200"]:_woff["biasbc"]],
                            wpk_d[:, _woff["w200"]:_woff["biasbc"]])
        nc.sync.dma_start(wpk[:, _woff["biasbc"]:WCOLS],
                          wpk_d[:, _woff["biasbc"]:WCOLS])
        for c in range(3):
            nc.sync.dma_start(wlg_sb[c][:], wlg_d[c])

        # round-robin psum-drain helper across DVE / Act (Pool cannot
        # access PSUM on real hardware)
        _rr = [0]
        def rr_copy(dst, src):
            k = _rr[0] % 2
            _rr[0] += 1
            if k == 0:
                nc.vector.tensor_copy(dst, src)
            else:
                nc.scalar.activation(dst, src, AF.Copy, bias=0.0, scale=1.0)

        _ra = [0]
        def rr_add(dst, a, b):
            # PSUM-sourced adds: DVE only (Act cannot add two tensors,
            # Pool cannot touch PSUM)
            nc.vector.tensor_tensor(dst, a, b, op=ALU.add)

        # ------------- layernorm (token-major, bf16 out) -------------
        def layernorm_std(x_in, tag):
            """standardize (x-mu)*rstd over d per (t,b); in f32 [128, B*D],
            out bf16 [128, B*D]"""
            ssum = work.tile([128, B], F32, tag="ln_ss", name=f"{tag}_ss")
            nc.vector.tensor_reduce(ssum[:], x_in.rearrange("p (b d) -> p b d", d=D),
                                    axis=AX.X, op=ALU.add)
            mun = work.tile([128, B], F32, tag="ln_mu", name=f"{tag}_mu")
            nc.vector.tensor_scalar_mul(mun[:], ssum[:], 1.0 / D)
            sq = work.tile([128, B * D], F32, tag="ln_sq", name=f"{tag}_sq")
            nc.scalar.activation(sq[:], x_in, AF.Square, bias=0.0, scale=1.0)
            ssq = work.tile([128, B], F32, tag="ln_s2", name=f"{tag}_s2")
            nc.vector.tensor_reduce(ssq[:], sq[:].rearrange("p (b d) -> p b d", d=D),
                                    axis=AX.X, op=ALU.add)
            # var = ssq/D - mu^2 ; rstd = (var + eps)^-0.5
            m2 = work.tile([128, B], F32, tag="ln_m2", name=f"{tag}_m2")
            nc.gpsimd.tensor_tensor(m2[:], mun[:], mun[:], op=ALU.mult)
            v1 = work.tile([128, B], F32, tag="ln_v1", name=f"{tag}_v1")
            nc.vector.tensor_scalar(v1[:], ssq[:], 1.0 / D, 1e-5,
                                    op0=ALU.mult, op1=ALU.add)
            v2 = work.tile([128, B], F32, tag="ln_v2", name=f"{tag}_v2")
            nc.vector.tensor_tensor(v2[:], v1[:], m2[:], op=ALU.subtract)
            lnv = work.tile([128, B], F32, tag="ln_lv", name=f"{tag}_lv")
            nc.scalar.activation(lnv[:], v2[:], AF.Ln, bias=0.0, scale=1.0)
            rstd = work.tile([128, B], F32, tag="ln_rs", name=f"{tag}_rs")
            nc.scalar.activation(rstd[:], lnv[:], AF.Exp, bias=0.0, scale=-0.5)
            xn = work.tile([128, B * D], BF16, tag="ln_xn", name=f"{tag}_xn",
                           bufs=1)
            for b in range(B):
                nc.vector.tensor_scalar(xn[:, b * D:(b + 1) * D],
                                        x_in[:, b * D:(b + 1) * D],
                                        mun[:, b:b + 1], rstd[:, b:b + 1],
                                        op0=ALU.subtract, op1=ALU.mult)
            return xn

        def transpose_to(pool, xn, dest_tiles):
            """xn bf16 [128 t, (b, 256)] -> dest_tiles[c] bf16 [128 d, (b, t)]"""
            for c in range(2):
                pt = pool.tile([128, 512], BF16, tag="tp", name="tp", bufs=2)
                for b in range(B):
                    nc.tensor.transpose(pt[:, b * 128:(b + 1) * 128],
                                        xn[:, b * D + c * 128: b * D + (c + 1) * 128],
                                        idb_sb)
                rr_copy(dest_tiles[c][:], pt[:])

        # controller-state transposed C^T chunks [128, (b,t)=512]
        ct = [pers.tile([128, BT], BF16, tag=f"ct{c}", name=f"ct{c}") for c in range(3)]

        # ------------- transformer layers -------------
        for l in range(L):
            h1 = layernorm_std(x_sb, f"ln1_{l}")
            h1t = [work.tile([128, BT], BF16, tag=f"ht{c}", name=f"ht{c}", bufs=1)
                   for c in range(2)]
            # per-head q/k [32, (b,t)] at base partition 0 (PE requires
            # uniform base-0 operands within a psum accumulation bank)
            q_bf = [work.tile([32, BT], BF16, tag=f"qh{h}", name=f"qh{h}", bufs=1)
                    for h in range(NH)]
            k_bf = [work.tile([32, BT], BF16, tag=f"kh{h}", name=f"kh{h}", bufs=1)
                    for h in range(NH)]
            v_sb = [work.tile([128, 512], BF16, tag=f"vb{p}", name=f"vb{p}", bufs=1)
                    for p in range(2)]
            with tc.tile_pool(name=f"psA{l}", bufs=1, space="PSUM") as psA:
                transpose_to(psA, h1, h1t)
                for m in range(4):
                    qkp = psA.tile([128, BT], F32, tag="qkp", name="qkp", bufs=2)
                    for c in range(2):
                        nc.tensor.matmul(qkp[:], wqk_sb[l][c][:, m * 128:(m + 1) * 128],
                                         h1t[c][:], start=(c == 0), stop=(c == 1))
                    for j in range(4):
                        h = (m % 2) * 4 + j
                        dst = (q_bf if m < 2 else k_bf)[h]
                        rr_copy(dst[:], qkp[j * 32:(j + 1) * 32, :])
                # v time-major bf16, 2 samples per [128, 512] tile
                for p in range(2):
                    vp = psA.tile([128, 512], F32, tag="vp", name="vp", bufs=2)
                    for b2 in range(2):
                        b = p * 2 + b2
                        for c in range(2):
                            nc.tensor.matmul(vp[:, b2 * D:(b2 + 1) * D],
                                             h1t[c][:, b * 128:(b + 1) * 128],
                                             wvw_sb[l][c], start=(c == 0), stop=(c == 1))
                    rr_copy(v_sb[p][:], vp[:])

            # attention
            ot_sb = [work.tile([128, BT], BF16, tag=f"ot{mt}", name=f"ot{mt}", bufs=1)
                     for mt in range(2)]
            with tc.tile_pool(name=f"psB{l}", bufs=1, space="PSUM") as psB:
                for mt in range(2):
                    otp = psB.tile([128, BT], F32, tag="otp", name="otp", bufs=2)
                    for b in range(B):
                        scp = psB.tile([128, 512], F32, tag="scp", name="scp", bufs=2)
                        for j in range(4):
                            h = mt * 4 + j
                            cols = slice(b * 128, (b + 1) * 128)
                            nc.tensor.matmul(scp[:, j * 128:(j + 1) * 128],
                                             q_bf[h][:, cols], k_bf[h][:, cols],
                                             start=True, stop=True)
                        att = work.tile([128, 512], BF16, tag="att", name="att")
                        nc.scalar.activation(att[:], scp[:], AF.Exp, bias=0.0, scale=1.0)
                        amask = work.tile([128, 512], BF16, tag="amask", name="amask")
                        nc.vector.tensor_tensor(amask[:], att[:], mask4_sb, op=ALU.mult)
                        rs = work.tile([128, 4], F32, tag="rsum", name="rsum")
                        nc.vector.tensor_reduce(rs[:],
                                                amask[:].rearrange("p (j t) -> p j t", t=128),
                                                axis=AX.X, op=ALU.add)
                        rr = work.tile([128, 4], F32, tag="rrec", name="rrec")
                        nc.vector.reciprocal(rr[:], rs[:])
                        attn = work.tile([128, 512], BF16, tag="attn", name="attn")
                        for j in range(4):
                            dst = attn[:, j * 128:(j + 1) * 128]
                            src = amask[:, j * 128:(j + 1) * 128]
                            if j == 0:
                                nc.scalar.activation(dst, src, AF.Copy, bias=0.0,
                                                     scale=rr[:, j:j + 1])
                            elif j == 1:
                                nc.gpsimd.tensor_scalar_mul(dst, src, rr[:, j:j + 1])
                            else:
                                nc.vector.tensor_scalar_mul(dst, src, rr[:, j:j + 1])
                        atp = psB.tile([128, 512], BF16, tag="atp", name="atp", bufs=2)
                        for j in range(4):
                            nc.tensor.transpose(atp[:, j * 128:(j + 1) * 128],
                                                attn[:, j * 128:(j + 1) * 128], idb_sb)
                        attb = work.tile([128, 512], BF16, tag="attb", name="attb")
                        rr_copy(attb[:], atp[:])
                        for j in range(4):
                            h = mt * 4 + j
                            vs = v_sb[b // 2][:, (b % 2) * D + h * 32:(b % 2) * D + h * 32 + 32]
                            nc.tensor.matmul(otp[j * 32:(j + 1) * 32, b * 128:(b + 1) * 128],
                                             vs, attb[:, j * 128:(j + 1) * 128],
                                             start=True, stop=True,
                                             tile_position=(0, j * 32))
                    rr_copy(ot_sb[mt][:], otp[:])
            with tc.tile_pool(name=f"psW{l}", bufs=1, space="PSUM") as psW:
                for p in range(2):
                    yp = psW.tile([128, 512], F32, tag="yp", name="yp", bufs=2)
                    for b2 in range(2):
                        b = p * 2 + b2
                        for c in range(2):
                            nc.tensor.matmul(yp[:, b2 * D:(b2 + 1) * D],
                                             ot_sb[c][:, b * 128:(b + 1) * 128],
                                             wo_sb[l][c], start=(c == 0), stop=(c == 1))
                    nc.vector.tensor_tensor(x_sb[:, p * 512:(p + 1) * 512],
                                            x_sb[:, p * 512:(p + 1) * 512], yp[:],
                                            op=ALU.add)

            # FF
            h2 = layernorm_std(x_sb, f"ln2_{l}")
            h2t = [work.tile([128, BT], BF16, tag=f"ht{c}", name=f"ht{c}", bufs=1)
                   for c in range(2)]
            ut = [work.tile([128, BT], BF16, tag=f"ut{ft}", name=f"ut{ft}", bufs=1)
                  for ft in range(8)]
            with tc.tile_pool(name=f"psC{l}", bufs=1, space="PSUM") as psC:
                transpose_to(psC, h2, h2t)
                for ft in range(8):
                    up = psC.tile([128, BT], F32, tag="up", name="up", bufs=3)
                    for c in range(2):
                        nc.tensor.matmul(up[:], w1_sb[l][c][:, ft * 128:(ft + 1) * 128],
                                         h2t[c][:], start=(c == 0), stop=(c == 1))
                    # gelu: ut = uu*(1+tanh(0.79788456*(uu + 0.044715 uu^3)))
                    # (0.5 folded into W2); uu = up + b1
                    uu = work.tile([128, BT], BF16, tag="gl_u", name="gl_u", bufs=2)
                    if ft % 2 == 0:
                        nc.vector.tensor_scalar_add(uu[:], up[:], b1c_sb[l][:, ft:ft + 1])
                    else:
                        nc.scalar.activation(uu[:], up[:], AF.Identity,
                                             bias=b1c_sb[l][:, ft:ft + 1], scale=1.0)
                    u2 = work.tile([128, BT], BF16, tag="gl_u2", name="gl_u2", bufs=2)
                    nc.vector.tensor_tensor(u2[:], uu[:], uu[:], op=ALU.mult)
                    gq = work.tile([128, BT], BF16, tag="gl_q", name="gl_q", bufs=2)
                    nc.gpsimd.tensor_scalar(gq[:], u2[:], 0.044715, 1.0,
                                            op0=ALU.mult, op1=ALU.add)
                    gb = work.tile([128, BT], BF16, tag="gl_b", name="gl_b", bufs=2)
                    nc.vector.tensor_tensor(gb[:], gq[:], uu[:], op=ALU.mult)
                    gt = work.tile([128, BT], BF16, tag="gl_t", name="gl_t", bufs=2)
                    nc.scalar.activation(gt[:], gb[:], AF.Tanh, bias=0.0,
                                         scale=0.7978845608028654)
                    nc.vector.scalar_tensor_tensor(ut[ft][:], gt[:], 1.0, uu[:],
                                                   op0=ALU.add, op1=ALU.mult)
                for p in range(2):
                    y2 = psC.tile([128, 512], F32, tag="y2p", name="y2p", bufs=2)
                    for b2 in range(2):
                        b = p * 2 + b2
                        for c in range(8):
                            nc.tensor.matmul(y2[:, b2 * D:(b2 + 1) * D],
                                             ut[c][:, b * 128:(b + 1) * 128],
                                             w2_sb[l][c], start=(c == 0), stop=False)
                        nc.tensor.matmul(y2[:, b2 * D:(b2 + 1) * D], ones_sb,
                                         b2_sb[l], start=False, stop=True)
                    nc.vector.tensor_tensor(x_sb[:, p * 512:(p + 1) * 512],
                                            x_sb[:, p * 512:(p + 1) * 512], y2[:],
                                            op=ALU.add)

        # final LN -> C^T chunks 0,1
        hn = layernorm_std(x_sb, "lnf")
        with tc.tile_pool(name="psF", bufs=1, space="PSUM") as psF:
            transpose_to(psF, hn, ct)
        nc.vector.memset(ct[2][:], 0.0)

        # Lh: logits h-part accumulator (bf16), per b [128, VSH]
        Lh = [pers.tile([128, VSH], BF16, tag=f"Lh{b}", name=f"Lh{b}") for b in range(B)]
        obuf = [pers.tile([128, VSH], BF16, tag=f"ob{b}", name=f"ob{b}") for b in range(B)]

        # ------------- recurrence fixed point, logits h-part interleaved ----
        K_IT = K_ITERS if PH >= 2 else 0
        hblocks = [(b, vt) for b in range(B) for vt in range(NVT)]
        hb_per_iter = (len(hblocks) + K_IT - 1) // max(K_IT, 1) if PH >= 3 else 0

        with tc.tile_pool(name="psR", bufs=1, space="PSUM") as psR, \
             tc.tile_pool(name="psL", bufs=1, space="PSUM") as psL:

            def emit_hblock(b, vt):
                # logits h-part: chunk0+chunk1 (+ bias) into Lh[b] (bf16)
                kk = b * NVT + vt
                vcols = slice(vt * VT, (vt + 1) * VT)
                lg = psL.tile([128, VT], F32, tag="lg", name="lg", bufs=4)
                if kk % 2 == 0:
                    for c in range(2):
                        nc.tensor.matmul(lg[:], ct[c][:, b * 128:(b + 1) * 128],
                                         wlg_sb[c][:, vcols], start=(c == 0), stop=(c == 1))
                    nc.vector.tensor_tensor(Lh[b][:, vcols], lg[:],
                                            biasbc_sb[:, vcols], op=ALU.add)
                else:
                    # bias via ones (x) blg-row matmul, drain as Act copy
                    for c in range(2):
                        nc.tensor.matmul(lg[:], ct[c][:, b * 128:(b + 1) * 128],
                                         wlg_sb[c][:, vcols], start=(c == 0), stop=False)
                    nc.tensor.matmul(lg[:], ones_sb, biasbc_sb[0:1, vcols],
                                     start=False, stop=True)
                    nc.scalar.activation(Lh[b][:, vcols], lg[:], AF.Copy,
                                         bias=0.0, scale=1.0)

            hbi = 0
            for it in range(K_IT):
                nchunk = 2 if it == 0 else 3
                wvp = psR.tile([128, BT], F32, tag="wvp", name="wvp", bufs=1)
                erp = psR.tile([128, BT], F32, tag="erp", name="erp", bufs=1)
                gp = psR.tile([4, BT], F32, tag="gp", name="gp", bufs=1)
                for c in range(nchunk):
                    st, sp = (c == 0), (c == nchunk - 1)
                    nc.tensor.matmul(erp[:], wifc_sb[c][:, 128:256], ct[c][:], start=st, stop=sp)
                    nc.tensor.matmul(wvp[:], wifc_sb[c][:, 0:128], ct[c][:], start=st, stop=sp)
                    nc.tensor.matmul(gp[:], wifc_sb[c][:, 256:260], ct[c][:], start=st, stop=sp)
                se = work.tile([128, BT], F32, tag="se", name="se", bufs=1)
                nc.scalar.activation(se[:], erp[:], AF.Sigmoid, bias=ber_sb[:, 0:1], scale=1.0)
                sa = work.tile([4, BT], BF16, tag="sa", name="sa")
                nc.scalar.activation(sa[:], gp[:], AF.Sigmoid, bias=bg_sb[:, 0:1], scale=1.0)
                # s = 1 - se/512, zeroed at batch starts (segmented scan reset)
                s_sb = work.tile([128, BT], F32, tag="ssb", name="ssb", bufs=1)
                nc.gpsimd.tensor_scalar(s_sb[:], se[:], -1.0 / SLOTS, 1.0,
                                        op0=ALU.mult, op1=ALU.add)
                for b in range(B):
                    nc.vector.memset(s_sb[:, b * 128:b * 128 + 1], 0.0)
                wvb = work.tile([128, BT], F32, tag="wvb", name="wvb", bufs=1)
                nc.vector.tensor_scalar_add(wvb[:], wvp[:], bwv_sb[:, 0:1])
                sabc = psR.tile([128, BT], F32, tag="sabc", name="sabc", bufs=1)
                nc.tensor.matmul(sabc[:], blk_sb, sa[:], start=True, stop=True)
                u_sb = work.tile([128, BT], F32, tag="usb", name="usb", bufs=1)
                nc.vector.tensor_tensor(u_sb[:], wvb[:], sabc[:], op=ALU.mult)
                # m_t = s_t*m_{t-1} + u_t   (segmented via zeroed s at starts)
                msc = work.tile([128, BT], F32, tag="msc", name="msc", bufs=1)
                nc.vector.tensor_tensor_scan(msc[:], s_sb[:], u_sb[:], 0.0,
                                             op0=ALU.mult, op1=ALU.add)
                # prefix sum of m (segmented via bmask)
                pcs = work.tile([128, BT], F32, tag="pcs", name="pcs", bufs=1)
                nc.vector.tensor_tensor_scan(pcs[:], bmaskf_sb, msc[:], 0.0,
                                             op0=ALU.mult, op1=ALU.add)
                for b in range(B):
                    nc.gpsimd.tensor_copy(ct[2][:, b * 128 + 2:(b + 1) * 128],
                                          pcs[:, b * 128: b * 128 + 126])
                # interleave logits h-part on PE behind this iteration
                if PH >= 3:
                    for _ in range(hb_per_iter):
                        if hbi < len(hblocks):
                            emit_hblock(*hblocks[hbi])
                            hbi += 1
            while PH >= 3 and hbi < len(hblocks):
                emit_hblock(*hblocks[hbi])
                hbi += 1

        # ------------- logits rv-part + output -------------
        if PH < 3:
            xdump = pers.tile([128, B * D], BF16, tag="xdump", name="xdump")
            nc.vector.tensor_copy(xdump[:], x_sb)
            nc.sync.dma_start(out_d[0, :, 0:B * D], xdump[:])
            nc.sync.dma_start(out_d[1, :, 0:BT], ct[2][:])
        else:
            with tc.tile_pool(name="psL2", bufs=1, space="PSUM") as psL2:
                for b in range(B):
                    for vt in range(NVT):
                        kk = b * NVT + vt
                        vcols = slice(vt * VT, (vt + 1) * VT)
                        lg2 = psL2.tile([128, VT], F32, tag="lg2", name="lg2", bufs=4)
                        nc.tensor.matmul(lg2[:], ct[2][:, b * 128:(b + 1) * 128],
                                         wlg_sb[2][:, vcols], start=True, stop=True)
                        if kk % 2 == 0:
                            nc.vector.tensor_tensor(obuf[b][:, vcols], lg2[:],
                                                    Lh[b][:, vcols], op=ALU.add)
                        else:
                            lt = work.tile([128, VT], BF16, tag="lt", name="lt", bufs=2)
                            nc.scalar.activation(lt[:], lg2[:], AF.Copy,
                                                 bias=0.0, scale=1.0)
                            nc.vector.tensor_tensor(obuf[b][:, vcols], lt[:],
                                                    Lh[b][:, vcols], op=ALU.add)
                    half = VSH // 2
                    if b % 2 == 0:
                        nc.sync.dma_start(out_d[b, :, 0:half], obuf[b][:, 0:half])
                        nc.scalar.dma_start(out_d[b, :, half:VSH], obuf[b][:, half:VSH])
                    else:
                        nc.gpsimd.dma_start(out_d[b, :, 0:half], obuf[b][:, 0:half])
                        nc.sync.dma_start(out_d[b, :, half:VSH], obuf[b][:, half:VSH])

    _split_excess_waits(nc, maxw=1)
    return nc


def _split_excess_waits(nc, maxw=1):
    """The TPB instruction encodings carry a limited number of sync-wait
    slots; walrus rejects instructions with more ("Too many sync wait
    commands").  Move overflow waits onto same-engine NoOps inserted just
    before the instruction (program order on the sequencer preserves the
    blocking semantics)."""
    for fn in nc.m.functions:
        for blk in fn.blocks:
            insts = blk.instructions
            idx = 0
            while idx < len(insts):
                ins = insts[idx]
                si = ins.sync_info
                if si is not None and len(si.on_wait) > maxw:
                    waits = list(si.on_wait)
                    keep = waits[-maxw:]
                    overflow = waits[:-maxw]
                    for j in range(0, len(overflow), max(maxw, 1)):
                        chunk = overflow[j:j + max(maxw, 1)]
                        nop = mybir.InstNoOp(name=nc.get_next_instruction_name(), ins=[], outs=[])
                        nop.engine = ins.engine
                        nop.sync_info = mybir.SyncInfo(on_wait=chunk, on_update=[])
                        nc.register_instruction(nop)
                        insts.insert(idx, nop)
                        idx += 1
                    si.on_wait = keep
                idx += 1


# ---------------- host side ----------------
_NC_CACHE = {}


def _get_nc():
    if "nc" not in _NC_CACHE:
        _NC_CACHE["nc"] = build_nc()
    return _NC_CACHE["nc"]


def prepare_in_maps(input_seq, tok_emb, pos_emb, Wqkv, Wo, ln1_g, ln1_b, ln2_g, ln2_b,
                    W1, b1, W2, b2, lnf_g, lnf_b, W_logits, b_logits, W_iface, b_iface,
                    beta_read, beta_write):
    f = np.float32
    bf = ml_dtypes.bfloat16
    input_seq = np.asarray(input_seq)
    tok_emb = np.asarray(tok_emb, f)
    pos_emb = np.asarray(pos_emb, f)
    Wqkv = np.asarray(Wqkv, f); Wo = np.asarray(Wo, f)
    ln1_g = np.asarray(ln1_g, f); ln1_b = np.asarray(ln1_b, f)
    ln2_g = np.asarray(ln2_g, f); ln2_b = np.asarray(ln2_b, f)
    W1 = np.asarray(W1, f); b1 = np.asarray(b1, f)
    W2 = np.asarray(W2, f); b2 = np.asarray(b2, f)
    lnf_g = np.asarray(lnf_g, f); lnf_b = np.asarray(lnf_b, f)
    W_logits = np.asarray(W_logits, f); b_logits = np.asarray(b_logits, f)
    W_iface = np.asarray(W_iface, f); b_iface = np.asarray(b_iface, f)

    # embedding (input prep)
    x0 = (tok_emb[input_seq] + pos_emb[:T]).astype(f)            # [B, T, D]
    x0 = np.ascontiguousarray(x0.transpose(1, 0, 2).reshape(T, B * D))

    # LN gamma folds (beta folds where a bias path exists)
    wqk = np.ascontiguousarray((ln1_g[:, :, None] * Wqkv[:, :, :2 * D]))
    wqk[:, :, :D] *= f(1.0 / np.sqrt(HD_ATT))
    wqk = wqk.reshape(L, 2, 128, 2 * D)
    wvw = (ln1_g[:, :, None] * Wqkv[:, :, 2 * D:]).reshape(L, 2, 128, D)
    wo_r = Wo.reshape(L, 2, 128, D)
    w1 = (ln2_g[:, :, None] * W1).reshape(L, 2, 128, FF)
    b1c = np.ascontiguousarray(
        (b1 + np.einsum("ld,ldf->lf", ln2_b, W1)).reshape(L, 8, 128).transpose(0, 2, 1))
    w2 = (0.5 * W2).reshape(L, 8, 128, D)

    # W_iface columns: per head block h*(4*HD+1): [rk, wk, wv, er, gate]
    Wif = W_iface.copy()
    Wif[:D] *= lnf_g[:, None]
    bif_full = b_iface + lnf_b @ W_iface[:D]
    cols_wv, cols_er, cols_g = [], [], []
    for h in range(MH):
        base = h * (4 * HD + 1)
        cols_wv += list(range(base + 2 * HD, base + 3 * HD))
        cols_er += list(range(base + 3 * HD, base + 4 * HD))
        cols_g.append(base + 4 * HD)
    wifc = np.ascontiguousarray(
        np.concatenate([Wif[:, cols_wv] / SLOTS, Wif[:, cols_er], Wif[:, cols_g]], axis=1)
    ).reshape(3, 128, 260)
    bwv = (bif_full[cols_wv] / SLOTS).reshape(128).astype(f)
    ber = bif_full[cols_er].reshape(128).astype(f)
    bg = bif_full[cols_g].reshape(4).astype(f)

    Wlg = W_logits.copy()
    Wlg[:D] *= lnf_g[:, None]
    blg_full = (b_logits + lnf_b @ W_logits[:D]).astype(f)

    mask1 = np.tril(np.ones((T, T), f))
    mask4 = np.tile(mask1, (1, 4))
    bmask = np.ones((128, 512), f)
    bmask[:, ::128] = 0.0
    idb = np.eye(128, dtype=f)
    ones_blk = np.zeros((128, 128), f)
    ones_blk[0, :] = 1.0
    blk = np.zeros((128, 128), f)
    for h in range(MH):
        blk[h, h * HD:(h + 1) * HD] = 1.0

    # ---- assemble packs ----
    wpack = np.zeros((128, WCOLS), f)
    def put(name, arr):
        off = _woff[name]
        r, c = arr.shape
        wpack[:r, off:off + c] = arr
    for l in range(L):
        for c in range(2):
            put(f"wqk{l}{c}", wqk[l, c])
            put(f"wv{l}{c}", wvw[l, c])
            put(f"wo{l}{c}", wo_r[l, c])
            put(f"w1{l}{c}", w1[l, c])
        for c in range(8):
            put(f"w2{l}{c}", w2[l, c])
        put(f"b2{l}", b2[l].reshape(1, D))
    for c in range(3):
        put(f"wifc{c}", wifc[c])
    put("mask4", mask4)
    put("bmask", bmask)
    put("idb", idb)
    put("ones1", ones_blk)
    put("blk", blk)

    fpack = np.zeros((128, FCOLS), f)
    fpack[:, _foff["x0"]:_foff["x0"] + B * D] = x0
    for l in range(L):
        fpack[:, _foff[f"b1c{l}"]:_foff[f"b1c{l}"] + 8] = b1c[l]
    fpack[:, _foff["bmaskf"]:_foff["bmaskf"] + 512] = bmask
    fpack[:, _foff["bwv"]] = bwv
    fpack[:, _foff["ber"]] = ber
    fpack[0:4, _foff["bg"]] = bg

    in_maps = []
    for cc in range(NCORES):
        sl = slice(cc * VSH, (cc + 1) * VSH)
        wp = wpack.copy()
        wp[:, _woff["biasbc"]:_woff["biasbc"] + VSH] = blg_full[sl][None, :]
        m = {
            "fpk": fpack,
            "wpk": wp.astype(bf),
            "wlg": np.ascontiguousarray(Wlg[:, sl]).reshape(3, 128, VSH).astype(bf),
        }
        in_maps.append(m)
    return in_maps


def kernel(**inputs):
    in_maps = prepare_in_maps(**inputs)
    nc = _get_nc()
    res = run_bass_kernel_spmd(nc, in_maps, list(range(NCORES))).results
    out = np.concatenate([np.asarray(res[c]["out"]) for c in range(NCORES)], axis=-1)
    return out.astype(np.float32)
